# revision 27
# baseline (speedup 1.0000x reference)
"""Bidirectional GRU on 8 Trainium2 NeuronCores.

Problem: x (128, 512, 256), GRU hidden 512, two directions, column-norm
constrained Wh gate blocks. Sharding: 2 directions x 4 batch slices of 32.
All 8 cores run an identical SPMD program (forward GRU); the backward
direction is realized by time-flipping x on the host for cores 4-7 and
un-flipping their output states.

Kernel layout (per core): state is kept transposed (hT: H on partitions,
batch on the free axis) so the recurrent matmuls use the resident Wh blocks
as stationary operands and no per-step transposes are needed.
Phase 1 precomputes xpT = Wx^T @ xT + b for all steps (bf16, PE) into DRAM;
phase 2 runs the 512-step recurrence with fp32 state and bf16 matmul
operands.
"""

import os

import numpy as np
import ml_dtypes

B, T, I, H = 128, int(os.environ.get("GRU_T", "512")), 256, 512
G3 = 3 * H  # 1536
N_CORES = 8
N_BSHARD = 4
BL = B // N_BSHARD  # 32 batch per core
MAX_NORM = 1.0

BF16 = ml_dtypes.bfloat16

_prog_cache = {}
last_exec_time_ns = None


# ---------------------------------------------------------------------------
# Tile drain patch: this container's walrus build rejects instructions that
# carry too many sem waits ("Too many sync wait commands" in CoreV3 codegen).
# Spread the final Tile drain's waits across preceding sync-engine nops.
# ---------------------------------------------------------------------------
def _install_tile_drain_patch():
    import concourse.mybir as mybir
    import concourse.tile as tile
    from concourse.vector_clock import ScopedClock

    if getattr(tile.TileContext, "_drain_patch_installed", False):
        return

    max_waits = 1

    def _patched_drain_and_barrier(self, tick_clock, wait_clock):
        nc = self.nc
        lead = nc.sync.nop(nofuse=True)
        wait_clock.add_sem_waits(
            lead.ins, ScopedClock({None: tick_clock.global_clock})
        )
        si = lead.ins.sync_info
        waits = list(si.on_wait or []) if si is not None else []
        if len(waits) > max_waits:
            si.on_wait = waits[:max_waits]
            for i in range(max_waits, len(waits), max_waits):
                nop = nc.sync.nop(nofuse=True)
                nop.ins.sync_info = mybir.SyncInfo(
                    on_wait=waits[i : i + max_waits], on_update=[]
                )
        nc.sync.drain()

        nc.all_engine_barrier()
        assert self.sems is not None
        popped = nc._tile_sem_poison_stack.pop()
        assert popped is self._sem_poison
        nc.clear_and_free_semaphores(list(self.sems.allocated().values()))
        nc.all_engine_barrier()

    tile.TileContext._drain_and_barrier = _patched_drain_and_barrier
    tile.TileContext._drain_patch_installed = True


def _split_multiwait_json(bir_bytes, max_waits=1):
    """This walrus build rejects instructions carrying more than one sem wait.
    Hoist extra waits onto same-engine NoOps inserted just before the
    instruction (engines execute their queue in order, so semantics hold)."""
    import json

    bir = json.loads(bir_bytes)
    ctr = 0
    for f in bir["functions"]:
        for blk in f["blocks"]:
            out = []
            for inst in blk["instructions"]:
                si = inst.get("sync_info")
                ow = (si or {}).get("on_wait") or []
                if len(ow) > max_waits:
                    extra = ow[: len(ow) - max_waits]
                    for i in range(0, len(extra), max_waits):
                        ctr += 1
                        out.append(
                            {
                                "debug": 0,
                                "engine": inst["engine"],
                                "ins": [],
                                "outs": [],
                                "name": f"I-wn{ctr}",
                                "opcode": "NoOp",
                                "sync_info": {
                                    "on_update": [],
                                    "on_wait": extra[i : i + max_waits],
                                },
                            }
                        )
                    si["on_wait"] = ow[len(ow) - max_waits :]
                out.append(inst)
            blk["instructions"] = out
    return json.dumps(bir).encode()


def _build_program():
    import concourse.bass as bass
    import concourse.mybir as mybir
    import concourse.tile as tile

    _install_tile_drain_patch()

    FDT = mybir.dt.float32
    BDT = mybir.dt.bfloat16
    ACT = mybir.ActivationFunctionType
    NTOK = T * BL  # 16384

    nc = bass.Bass()
    xT_d = nc.dram_tensor("xT", [I, NTOK], BDT, kind="ExternalInput")
    wx_d = nc.dram_tensor("wx", [I, G3], BDT, kind="ExternalInput")
    wh_d = nc.dram_tensor("wh", [H, G3], BDT, kind="ExternalInput")
    bias_d = nc.dram_tensor("bias", [G3], FDT, kind="ExternalInput")
    states_d = nc.dram_tensor("states", [T, 128, 4, BL], BDT, kind="ExternalOutput")

    with tile.TileContext(nc) as tc:
        with (
            tc.tile_pool(name="const", bufs=1) as const,
            tc.tile_pool(name="xtp", bufs=1) as xtp,
        ):
            wx_sb = const.tile([128, 2, G3], BDT)
            nc.sync.dma_start(
                out=wx_sb[:], in_=wx_d.rearrange("(kb p) m -> p kb m", p=128)
            )
            wh_sb = const.tile([128, 4, G3], BDT)
            nc.sync.dma_start(
                out=wh_sb[:], in_=wh_d.rearrange("(kb p) m -> p kb m", p=128)
            )
            xT_sb = xtp.tile([128, 2, NTOK], BDT)
            nc.sync.dma_start(
                out=xT_sb[:], in_=xT_d.rearrange("(kb p) n -> p kb n", p=128)
            )

            bias_sb = const.tile([128, 12], FDT)
            nc.sync.dma_start(
                out=bias_sb[:], in_=bias_d.rearrange("(m p) -> p m", p=128)
            )
            # broadcast bias tiles: bias_g[p, mb*32+b] = b[g*512+mb*128+p]
            bias_r = const.tile([128, 128], FDT, tag="bias_r")
            bias_z = const.tile([128, 128], FDT, tag="bias_z")
            bias_h = const.tile([128, 128], FDT, tag="bias_h")
            for bt, g0 in ((bias_z, 0), (bias_r, 4), (bias_h, 8)):
                nc.vector.memset(bt[:], 0.0)
                for mb in range(4):
                    sl = slice(mb * 32, mb * 32 + 32)
                    nc.vector.tensor_scalar_add(
                        bt[:, sl], bt[:, sl], bias_sb[:, g0 + mb : g0 + mb + 1]
                    )

            # ---------------- recurrence ------------------------------------
            # Per step, per gate slice mb: PSUM group = [x-proj (2 matmuls,
            # no state dep) then 4 recurrent matmuls]. r gate first so its
            # sigmoid/mul overlap the z matmuls; h~ after rh is ready.
            with (
                tc.tile_pool(name="state", bufs=3) as state,
                tc.tile_pool(name="stateb", bufs=3) as stateb,
                tc.tile_pool(name="work", bufs=3) as work,
                tc.tile_pool(name="p_r", bufs=2, space="PSUM") as p_r,
                tc.tile_pool(name="p_z", bufs=2, space="PSUM") as p_z,
                tc.tile_pool(name="p_h", bufs=2, space="PSUM") as p_h,
            ):
                hb = stateb.tile([128, 128], BDT, tag="hb")
                nc.vector.memset(hb[:], 0.0)

                def gate_mms(ps, g, rhs_state, t):
                    for mb in range(4):
                        o0 = mb * 32
                        wcol = g * H + mb * 128
                        for kx in range(2):
                            nc.tensor.matmul(
                                ps[:, o0 : o0 + 32],
                                lhsT=wx_sb[:, kx, wcol : wcol + 128],
                                rhs=xT_sb[:, kx, t * BL : t * BL + BL],
                                start=(kx == 0),
                                stop=False,
                            )
                        for kb in range(4):
                            nc.tensor.matmul(
                                ps[:, o0 : o0 + 32],
                                lhsT=wh_sb[:, kb, wcol : wcol + 128],
                                rhs=rhs_state[:, kb * 32 : kb * 32 + 32],
                                start=False,
                                stop=(kb == 3),
                            )

                for t in range(T):
                    ps_r = p_r.tile([128, 128], FDT, tag="ps_r")
                    ps_z = p_z.tile([128, 128], FDT, tag="ps_z")
                    ps_h = p_h.tile([128, 128], FDT, tag="ps_h")

                    gate_mms(ps_r, 1, hb, t)
                    gate_mms(ps_z, 0, hb, t)

                    r_pre = work.tile([128, 128], FDT, tag="r_pre")
                    sig_r = work.tile([128, 128], BDT, tag="sig_r")
                    rh = work.tile([128, 128], BDT, tag="rh")
                    nc.vector.tensor_add(r_pre[:], ps_r[:], bias_r[:])
                    nc.scalar.activation(sig_r[:], r_pre[:], ACT.Sigmoid)
                    nc.vector.tensor_mul(rh[:], sig_r[:], hb[:])

                    gate_mms(ps_h, 2, rh, t)

                    z_pre = work.tile([128, 128], FDT, tag="z_pre")
                    sig_z = work.tile([128, 128], BDT, tag="sig_z")
                    nc.vector.tensor_add(z_pre[:], ps_z[:], bias_z[:])
                    nc.scalar.activation(sig_z[:], z_pre[:], ACT.Sigmoid)

                    # h~ + blend in two 64-col halves: tanh of half 1
                    # overlaps blend of half 0 (ACT and DVE in parallel)
                    h_pre = work.tile([128, 128], FDT, tag="h_pre")
                    th = work.tile([128, 128], BDT, tag="th")
                    dd = work.tile([128, 128], BDT, tag="dd")
                    ee = work.tile([128, 128], BDT, tag="ee")
                    hb_new = stateb.tile([128, 128], BDT, tag="hb")
                    for hf in range(2):
                        sl = slice(hf * 64, hf * 64 + 64)
                        nc.vector.tensor_add(h_pre[:, sl], ps_h[:, sl], bias_h[:, sl])
                        nc.scalar.activation(th[:, sl], h_pre[:, sl], ACT.Tanh)
                        nc.vector.tensor_sub(dd[:, sl], th[:, sl], hb[:, sl])
                        nc.vector.tensor_mul(ee[:, sl], dd[:, sl], sig_z[:, sl])
                        nc.vector.tensor_add(hb_new[:, sl], hb[:, sl], ee[:, sl])

                    nc.sync.dma_start(
                        out=states_d[t],
                        in_=hb_new.rearrange("p (kb b) -> p kb b", kb=4),
                    )
                    hb = hb_new

    _raw_to_json = nc.to_json_bytes
    nc.to_json_bytes = lambda: _split_multiwait_json(_raw_to_json())
    return nc


def _get_program():
    if "nc" not in _prog_cache:
        _prog_cache["nc"] = _build_program()
    return _prog_cache["nc"]


def _constrain_np(Wm):
    # column-norm constraint per (H,H) gate block, matching reference
    out = np.empty_like(Wm)
    for g in range(3):
        Wg = Wm[:, g * H : (g + 1) * H]
        norm = np.linalg.norm(Wg, axis=0, keepdims=True)
        desired = np.minimum(norm, MAX_NORM)
        out[:, g * H : (g + 1) * H] = Wg * (desired / (1e-7 + norm))
    return out


def kernel(x, Wx_f, Wh_f, b_f, Wx_b, Wh_b, b_b):
    global last_exec_time_ns
    from concourse.bass_utils import run_bass_kernel_spmd

    x = np.asarray(x, dtype=np.float32)
    Whf_c = _constrain_np(np.asarray(Wh_f, dtype=np.float32))
    Whb_c = _constrain_np(np.asarray(Wh_b, dtype=np.float32))
    wx_f = np.asarray(Wx_f, dtype=BF16)
    wx_b = np.asarray(Wx_b, dtype=BF16)
    wh_f = Whf_c.astype(BF16)
    wh_b = Whb_c.astype(BF16)
    bf = np.asarray(b_f, dtype=np.float32)
    bb = np.asarray(b_b, dtype=np.float32)

    in_maps = []
    for c in range(N_CORES):
        d, s = divmod(c, N_BSHARD)
        xs = x[s * BL : (s + 1) * BL]
        if d == 1:
            xs = xs[:, ::-1]
        xTc = np.ascontiguousarray(xs.transpose(2, 1, 0)).reshape(I, T * BL)
        in_maps.append(
            {
                "xT": xTc.astype(BF16),
                "wx": wx_f if d == 0 else wx_b,
                "wh": wh_f if d == 0 else wh_b,
                "bias": bf if d == 0 else bb,
            }
        )

    nc = _get_program()
    trace = os.environ.get("BASS_TRACE") == "1"
    res = run_bass_kernel_spmd(nc, in_maps, list(range(N_CORES)), trace=trace)
    last_exec_time_ns = res.exec_time_ns

    def to_bth(st):
        # (T, 128, 4, BL) -> (BL, T, H) with H = kb*128 + p
        return np.ascontiguousarray(
            st.astype(np.float32).transpose(3, 0, 2, 1).reshape(BL, T, H)
        )

    fwd = np.concatenate([to_bth(res.results[c]["states"]) for c in range(4)], axis=0)
    bwd = np.concatenate(
        [to_bth(res.results[c + 4]["states"]) for c in range(4)], axis=0
    )[:, ::-1]
    combined = np.concatenate([fwd[:, -1], bwd[:, 0]], axis=-1)
    return (
        np.ascontiguousarray(fwd),
        np.ascontiguousarray(bwd),
        np.ascontiguousarray(combined),
    )


# revision 29
# speedup vs baseline: 1.0009x; 1.0009x over previous
"""Bidirectional GRU on 8 Trainium2 NeuronCores.

Problem: x (128, 512, 256), GRU hidden 512, two directions, column-norm
constrained Wh gate blocks. Sharding: 2 directions x 4 batch slices of 32.
All 8 cores run an identical SPMD program (forward GRU); the backward
direction is realized by time-flipping x on the host for cores 4-7 and
un-flipping their output states.

Kernel layout (per core): state is kept transposed (hT: H on partitions,
batch on the free axis) so the recurrent matmuls use the resident Wh blocks
as stationary operands and no per-step transposes are needed.
Phase 1 precomputes xpT = Wx^T @ xT + b for all steps (bf16, PE) into DRAM;
phase 2 runs the 512-step recurrence with fp32 state and bf16 matmul
operands.
"""

import os

import numpy as np
import ml_dtypes

B, T, I, H = 128, int(os.environ.get("GRU_T", "512")), 256, 512
G3 = 3 * H  # 1536
N_CORES = 8
N_BSHARD = 4
BL = B // N_BSHARD  # 32 batch per core
MAX_NORM = 1.0

BF16 = ml_dtypes.bfloat16

_prog_cache = {}
last_exec_time_ns = None


# ---------------------------------------------------------------------------
# Tile drain patch: this container's walrus build rejects instructions that
# carry too many sem waits ("Too many sync wait commands" in CoreV3 codegen).
# Spread the final Tile drain's waits across preceding sync-engine nops.
# ---------------------------------------------------------------------------
def _install_tile_drain_patch():
    import concourse.mybir as mybir
    import concourse.tile as tile
    from concourse.vector_clock import ScopedClock

    if getattr(tile.TileContext, "_drain_patch_installed", False):
        return

    max_waits = 1

    def _patched_drain_and_barrier(self, tick_clock, wait_clock):
        nc = self.nc
        lead = nc.sync.nop(nofuse=True)
        wait_clock.add_sem_waits(
            lead.ins, ScopedClock({None: tick_clock.global_clock})
        )
        si = lead.ins.sync_info
        waits = list(si.on_wait or []) if si is not None else []
        if len(waits) > max_waits:
            si.on_wait = waits[:max_waits]
            for i in range(max_waits, len(waits), max_waits):
                nop = nc.sync.nop(nofuse=True)
                nop.ins.sync_info = mybir.SyncInfo(
                    on_wait=waits[i : i + max_waits], on_update=[]
                )
        nc.sync.drain()

        nc.all_engine_barrier()
        assert self.sems is not None
        popped = nc._tile_sem_poison_stack.pop()
        assert popped is self._sem_poison
        nc.clear_and_free_semaphores(list(self.sems.allocated().values()))
        nc.all_engine_barrier()

    tile.TileContext._drain_and_barrier = _patched_drain_and_barrier
    tile.TileContext._drain_patch_installed = True


def _split_multiwait_json(bir_bytes, max_waits=1):
    """This walrus build rejects instructions carrying more than one sem wait.
    Hoist extra waits onto same-engine NoOps inserted just before the
    instruction (engines execute their queue in order, so semantics hold)."""
    import json

    bir = json.loads(bir_bytes)
    ctr = 0
    for f in bir["functions"]:
        for blk in f["blocks"]:
            out = []
            for inst in blk["instructions"]:
                si = inst.get("sync_info")
                ow = (si or {}).get("on_wait") or []
                if len(ow) > max_waits:
                    extra = ow[: len(ow) - max_waits]
                    for i in range(0, len(extra), max_waits):
                        ctr += 1
                        out.append(
                            {
                                "debug": 0,
                                "engine": inst["engine"],
                                "ins": [],
                                "outs": [],
                                "name": f"I-wn{ctr}",
                                "opcode": "NoOp",
                                "sync_info": {
                                    "on_update": [],
                                    "on_wait": extra[i : i + max_waits],
                                },
                            }
                        )
                    si["on_wait"] = ow[len(ow) - max_waits :]
                out.append(inst)
            blk["instructions"] = out
    return json.dumps(bir).encode()


def _build_program():
    import concourse.bass as bass
    import concourse.mybir as mybir
    import concourse.tile as tile

    _install_tile_drain_patch()

    FDT = mybir.dt.float32
    BDT = mybir.dt.bfloat16
    ACT = mybir.ActivationFunctionType
    NTOK = T * BL  # 16384

    nc = bass.Bass()
    xT_d = nc.dram_tensor("xT", [I, NTOK], BDT, kind="ExternalInput")
    wx_d = nc.dram_tensor("wx", [I, G3], BDT, kind="ExternalInput")
    wh_d = nc.dram_tensor("wh", [H, G3], BDT, kind="ExternalInput")
    bias_d = nc.dram_tensor("bias", [G3], FDT, kind="ExternalInput")
    states_d = nc.dram_tensor("states", [T, 128, 4, BL], BDT, kind="ExternalOutput")

    with tile.TileContext(nc) as tc:
        with (
            tc.tile_pool(name="const", bufs=1) as const,
            tc.tile_pool(name="xtp", bufs=1) as xtp,
        ):
            wx_sb = const.tile([128, 2, G3], BDT)
            nc.sync.dma_start(
                out=wx_sb[:], in_=wx_d.rearrange("(kb p) m -> p kb m", p=128)
            )
            wh_sb = const.tile([128, 4, G3], BDT)
            nc.sync.dma_start(
                out=wh_sb[:], in_=wh_d.rearrange("(kb p) m -> p kb m", p=128)
            )
            xT_sb = xtp.tile([128, 2, NTOK], BDT)
            nc.sync.dma_start(
                out=xT_sb[:], in_=xT_d.rearrange("(kb p) n -> p kb n", p=128)
            )

            bias_sb = const.tile([128, 12], FDT)
            nc.sync.dma_start(
                out=bias_sb[:], in_=bias_d.rearrange("(m p) -> p m", p=128)
            )
            # broadcast bias tiles: bias_g[p, mb*32+b] = b[g*512+mb*128+p]
            bias_r = const.tile([128, 128], FDT, tag="bias_r")
            bias_z = const.tile([128, 128], FDT, tag="bias_z")
            bias_h = const.tile([128, 128], FDT, tag="bias_h")
            for bt, g0 in ((bias_z, 0), (bias_r, 4), (bias_h, 8)):
                nc.vector.memset(bt[:], 0.0)
                for mb in range(4):
                    sl = slice(mb * 32, mb * 32 + 32)
                    nc.vector.tensor_scalar_add(
                        bt[:, sl], bt[:, sl], bias_sb[:, g0 + mb : g0 + mb + 1]
                    )

            # ---------------- recurrence ------------------------------------
            # Per step, per gate slice mb: PSUM group = [x-proj (2 matmuls,
            # no state dep) then 4 recurrent matmuls]. r gate first so its
            # sigmoid/mul overlap the z matmuls; h~ after rh is ready.
            with (
                tc.tile_pool(name="state", bufs=3) as state,
                tc.tile_pool(name="stateb", bufs=3) as stateb,
                tc.tile_pool(name="work", bufs=3) as work,
                tc.tile_pool(name="p_r", bufs=2, space="PSUM") as p_r,
                tc.tile_pool(name="p_z", bufs=2, space="PSUM") as p_z,
                tc.tile_pool(name="p_h", bufs=2, space="PSUM") as p_h,
            ):
                hb = stateb.tile([128, 128], BDT, tag="hb")
                nc.vector.memset(hb[:], 0.0)

                def gate_mms(ps, g, rhs_state, t):
                    for mb in range(4):
                        o0 = mb * 32
                        wcol = g * H + mb * 128
                        for kx in range(2):
                            nc.tensor.matmul(
                                ps[:, o0 : o0 + 32],
                                lhsT=wx_sb[:, kx, wcol : wcol + 128],
                                rhs=xT_sb[:, kx, t * BL : t * BL + BL],
                                start=(kx == 0),
                                stop=False,
                            )
                        for kb in range(4):
                            nc.tensor.matmul(
                                ps[:, o0 : o0 + 32],
                                lhsT=wh_sb[:, kb, wcol : wcol + 128],
                                rhs=rhs_state[:, kb * 32 : kb * 32 + 32],
                                start=False,
                                stop=(kb == 3),
                            )

                for t in range(T):
                    ps_r = p_r.tile([128, 128], FDT, tag="ps_r")
                    ps_z = p_z.tile([128, 128], FDT, tag="ps_z")
                    ps_h = p_h.tile([128, 128], FDT, tag="ps_h")

                    gate_mms(ps_r, 1, hb, t)
                    gate_mms(ps_z, 0, hb, t)

                    r_pre = work.tile([128, 128], FDT, tag="r_pre")
                    sig_r = work.tile([128, 128], BDT, tag="sig_r")
                    rh = work.tile([128, 128], BDT, tag="rh")
                    nc.vector.tensor_add(r_pre[:], ps_r[:], bias_r[:])
                    nc.scalar.activation(sig_r[:], r_pre[:], ACT.Sigmoid)
                    nc.vector.tensor_mul(rh[:], sig_r[:], hb[:])

                    gate_mms(ps_h, 2, rh, t)

                    z_pre = work.tile([128, 128], FDT, tag="z_pre")
                    sig_z = work.tile([128, 128], BDT, tag="sig_z")
                    nc.vector.tensor_add(z_pre[:], ps_z[:], bias_z[:])
                    nc.scalar.activation(sig_z[:], z_pre[:], ACT.Sigmoid)

                    # h~ + blend in two 64-col halves: tanh of half 1
                    # overlaps blend of half 0 (ACT and DVE in parallel)
                    h_pre = work.tile([128, 128], FDT, tag="h_pre")
                    th = work.tile([128, 128], BDT, tag="th")
                    dd = work.tile([128, 128], BDT, tag="dd")
                    ee = work.tile([128, 128], BDT, tag="ee")
                    hb_new = stateb.tile([128, 128], BDT, tag="hb")
                    for hf in range(2):
                        sl = slice(hf * 64, hf * 64 + 64)
                        nc.vector.tensor_add(h_pre[:, sl], ps_h[:, sl], bias_h[:, sl])
                        nc.scalar.activation(th[:, sl], h_pre[:, sl], ACT.Tanh)
                        nc.vector.tensor_sub(dd[:, sl], th[:, sl], hb[:, sl])
                        nc.vector.tensor_mul(ee[:, sl], dd[:, sl], sig_z[:, sl])
                        nc.vector.tensor_add(hb_new[:, sl], hb[:, sl], ee[:, sl])

                    nc.sync.dma_start(
                        out=states_d[t],
                        in_=hb_new.rearrange("p (kb b) -> p kb b", kb=4),
                    )
                    hb = hb_new

    _raw_to_json = nc.to_json_bytes
    nc.to_json_bytes = lambda: _split_multiwait_json(_raw_to_json())
    return nc


def _get_program():
    if "nc" not in _prog_cache:
        _prog_cache["nc"] = _build_program()
    return _prog_cache["nc"]


def _constrain_np(Wm):
    # column-norm constraint per (H,H) gate block, matching reference
    out = np.empty_like(Wm)
    for g in range(3):
        Wg = Wm[:, g * H : (g + 1) * H]
        norm = np.linalg.norm(Wg, axis=0, keepdims=True)
        desired = np.minimum(norm, MAX_NORM)
        out[:, g * H : (g + 1) * H] = Wg * (desired / (1e-7 + norm))
    return out


def kernel(x, Wx_f, Wh_f, b_f, Wx_b, Wh_b, b_b):
    global last_exec_time_ns
    from concourse.bass_utils import run_bass_kernel_spmd

    x = np.asarray(x, dtype=np.float32)
    Whf_c = _constrain_np(np.asarray(Wh_f, dtype=np.float32))
    Whb_c = _constrain_np(np.asarray(Wh_b, dtype=np.float32))
    wx_f = np.asarray(Wx_f, dtype=BF16)
    wx_b = np.asarray(Wx_b, dtype=BF16)
    wh_f = Whf_c.astype(BF16)
    wh_b = Whb_c.astype(BF16)
    bf = np.asarray(b_f, dtype=np.float32)
    bb = np.asarray(b_b, dtype=np.float32)

    in_maps = []
    for c in range(N_CORES):
        d, s = divmod(c, N_BSHARD)
        xs = x[s * BL : (s + 1) * BL]
        if d == 1:
            xs = xs[:, ::-1]
        xTc = np.ascontiguousarray(xs.transpose(2, 1, 0)).reshape(I, T * BL)
        in_maps.append(
            {
                "xT": xTc.astype(BF16),
                "wx": wx_f if d == 0 else wx_b,
                "wh": wh_f if d == 0 else wh_b,
                "bias": bf if d == 0 else bb,
            }
        )

    nc = _get_program()
    trace = os.environ.get("BASS_TRACE") == "1"
    res = run_bass_kernel_spmd(nc, in_maps, list(range(N_CORES)), trace=trace)
    last_exec_time_ns = res.exec_time_ns

    def to_bth(st):
        # (T, 128, 4, BL) -> (BL, T, H) with H = kb*128 + p
        return np.ascontiguousarray(
            st.astype(np.float32).transpose(3, 0, 2, 1).reshape(BL, T, H)
        )

    fwd = np.concatenate([to_bth(res.results[c]["states"]) for c in range(4)], axis=0)
    bwd = np.concatenate(
        [to_bth(res.results[c + 4]["states"]) for c in range(4)], axis=0
    )[:, ::-1]
    combined = np.concatenate([fwd[:, -1], bwd[:, 0]], axis=-1)
    return (
        np.ascontiguousarray(fwd),
        np.ascontiguousarray(bwd),
        np.ascontiguousarray(combined),
    )


# revision 30
# speedup vs baseline: 1.0258x; 1.0249x over previous
"""Bidirectional GRU on 8 Trainium2 NeuronCores.

Problem: x (128, 512, 256), GRU hidden 512, two directions, column-norm
constrained Wh gate blocks. Sharding: 2 directions x 4 batch slices of 32.
All 8 cores run an identical SPMD program (forward GRU); the backward
direction is realized by time-flipping x on the host for cores 4-7 and
un-flipping their output states.

Kernel layout (per core): state is kept transposed (hT: H on partitions,
batch on the free axis) so the recurrent matmuls use the resident Wh blocks
as stationary operands and no per-step transposes are needed.
Phase 1 precomputes xpT = Wx^T @ xT + b for all steps (bf16, PE) into DRAM;
phase 2 runs the 512-step recurrence with fp32 state and bf16 matmul
operands.
"""

import os

import numpy as np
import ml_dtypes

B, T, I, H = 128, int(os.environ.get("GRU_T", "512")), 256, 512
G3 = 3 * H  # 1536
N_CORES = 8
N_BSHARD = 4
BL = B // N_BSHARD  # 32 batch per core
MAX_NORM = 1.0

BF16 = ml_dtypes.bfloat16

_prog_cache = {}
last_exec_time_ns = None


# ---------------------------------------------------------------------------
# Tile drain patch: this container's walrus build rejects instructions that
# carry too many sem waits ("Too many sync wait commands" in CoreV3 codegen).
# Spread the final Tile drain's waits across preceding sync-engine nops.
# ---------------------------------------------------------------------------
def _install_tile_drain_patch():
    import concourse.mybir as mybir
    import concourse.tile as tile
    from concourse.vector_clock import ScopedClock

    if getattr(tile.TileContext, "_drain_patch_installed", False):
        return

    max_waits = 1

    def _patched_drain_and_barrier(self, tick_clock, wait_clock):
        nc = self.nc
        lead = nc.sync.nop(nofuse=True)
        wait_clock.add_sem_waits(
            lead.ins, ScopedClock({None: tick_clock.global_clock})
        )
        si = lead.ins.sync_info
        waits = list(si.on_wait or []) if si is not None else []
        if len(waits) > max_waits:
            si.on_wait = waits[:max_waits]
            for i in range(max_waits, len(waits), max_waits):
                nop = nc.sync.nop(nofuse=True)
                nop.ins.sync_info = mybir.SyncInfo(
                    on_wait=waits[i : i + max_waits], on_update=[]
                )
        nc.sync.drain()

        nc.all_engine_barrier()
        assert self.sems is not None
        popped = nc._tile_sem_poison_stack.pop()
        assert popped is self._sem_poison
        nc.clear_and_free_semaphores(list(self.sems.allocated().values()))
        nc.all_engine_barrier()

    tile.TileContext._drain_and_barrier = _patched_drain_and_barrier
    tile.TileContext._drain_patch_installed = True


def _split_multiwait_json(bir_bytes, max_waits=1):
    """This walrus build rejects instructions carrying more than one sem wait.
    Hoist extra waits onto same-engine NoOps inserted just before the
    instruction (engines execute their queue in order, so semantics hold)."""
    import json

    bir = json.loads(bir_bytes)
    ctr = 0
    for f in bir["functions"]:
        for blk in f["blocks"]:
            out = []
            for inst in blk["instructions"]:
                si = inst.get("sync_info")
                ow = (si or {}).get("on_wait") or []
                if len(ow) > max_waits:
                    extra = ow[: len(ow) - max_waits]
                    for i in range(0, len(extra), max_waits):
                        ctr += 1
                        out.append(
                            {
                                "debug": 0,
                                "engine": inst["engine"],
                                "ins": [],
                                "outs": [],
                                "name": f"I-wn{ctr}",
                                "opcode": "NoOp",
                                "sync_info": {
                                    "on_update": [],
                                    "on_wait": extra[i : i + max_waits],
                                },
                            }
                        )
                    si["on_wait"] = ow[len(ow) - max_waits :]
                out.append(inst)
            blk["instructions"] = out
    return json.dumps(bir).encode()


def _build_program():
    import concourse.bass as bass
    import concourse.mybir as mybir
    import concourse.tile as tile

    _install_tile_drain_patch()

    FDT = mybir.dt.float32
    BDT = mybir.dt.bfloat16
    ACT = mybir.ActivationFunctionType
    NTOK = T * BL  # 16384

    nc = bass.Bass()
    xT_d = nc.dram_tensor("xT", [I, NTOK], BDT, kind="ExternalInput")
    wx_d = nc.dram_tensor("wx", [I, G3], BDT, kind="ExternalInput")
    wh_d = nc.dram_tensor("wh", [H, G3], BDT, kind="ExternalInput")
    bias_d = nc.dram_tensor("bias", [G3], FDT, kind="ExternalInput")
    states_d = nc.dram_tensor("states", [T, 128, 4, BL], BDT, kind="ExternalOutput")

    with tile.TileContext(nc) as tc:
        with (
            tc.tile_pool(name="const", bufs=1) as const,
            tc.tile_pool(name="xtp", bufs=1) as xtp,
        ):
            wx_sb = const.tile([128, 2, G3], BDT)
            nc.sync.dma_start(
                out=wx_sb[:], in_=wx_d.rearrange("(kb p) m -> p kb m", p=128)
            )
            wh_sb = const.tile([128, 4, G3], BDT)
            nc.sync.dma_start(
                out=wh_sb[:], in_=wh_d.rearrange("(kb p) m -> p kb m", p=128)
            )
            xT_sb = xtp.tile([128, 2, NTOK], BDT)
            nc.sync.dma_start(
                out=xT_sb[:], in_=xT_d.rearrange("(kb p) n -> p kb n", p=128)
            )

            bias_sb = const.tile([128, 12], FDT)
            nc.sync.dma_start(
                out=bias_sb[:], in_=bias_d.rearrange("(m p) -> p m", p=128)
            )
            # broadcast bias tiles: bias_g[p, mb*32+b] = b[g*512+mb*128+p]
            bias_r = const.tile([128, 128], FDT, tag="bias_r")
            bias_z = const.tile([128, 128], FDT, tag="bias_z")
            bias_h = const.tile([128, 128], FDT, tag="bias_h")
            for bt, g0 in ((bias_z, 0), (bias_r, 4), (bias_h, 8)):
                nc.vector.memset(bt[:], 0.0)
                for mb in range(4):
                    sl = slice(mb * 32, mb * 32 + 32)
                    nc.vector.tensor_scalar_add(
                        bt[:, sl], bt[:, sl], bias_sb[:, g0 + mb : g0 + mb + 1]
                    )

            # ---------------- recurrence ------------------------------------
            # Per step, per gate slice mb: PSUM group = [x-proj (2 matmuls,
            # no state dep) then 4 recurrent matmuls]. r gate first so its
            # sigmoid/mul overlap the z matmuls; h~ after rh is ready.
            with (
                tc.tile_pool(name="state", bufs=3) as state,
                tc.tile_pool(name="stateb", bufs=3) as stateb,
                tc.tile_pool(name="work", bufs=3) as work,
                tc.tile_pool(name="p_r", bufs=2, space="PSUM") as p_r,
                tc.tile_pool(name="p_z", bufs=2, space="PSUM") as p_z,
                tc.tile_pool(name="p_h", bufs=2, space="PSUM") as p_h,
            ):
                hb = stateb.tile([128, 128], BDT, tag="hb")
                nc.vector.memset(hb[:], 0.0)

                def gate_mms(ps, g, rhs_state, t):
                    for mb in range(4):
                        o0 = mb * 32
                        wcol = g * H + mb * 128
                        for kx in range(2):
                            nc.tensor.matmul(
                                ps[:, o0 : o0 + 32],
                                lhsT=wx_sb[:, kx, wcol : wcol + 128],
                                rhs=xT_sb[:, kx, t * BL : t * BL + BL],
                                start=(kx == 0),
                                stop=False,
                            )
                        for kb in range(4):
                            nc.tensor.matmul(
                                ps[:, o0 : o0 + 32],
                                lhsT=wh_sb[:, kb, wcol : wcol + 128],
                                rhs=rhs_state[:, kb * 32 : kb * 32 + 32],
                                start=False,
                                stop=(kb == 3),
                            )

                for t in range(T):
                    ps_ra = p_r.tile([128, 64], FDT, tag="ps_ra")
                    ps_rb = p_r.tile([128, 64], FDT, tag="ps_rb")
                    ps_z = p_z.tile([128, 128], FDT, tag="ps_z")
                    ps_h = p_h.tile([128, 128], FDT, tag="ps_h")

                    # r gate in two PSUM tiles: the add/sigmoid/mul chain of
                    # half A pipelines with half B's matmuls (per-tile deps)
                    r_pre = work.tile([128, 128], FDT, tag="r_pre")
                    sig_r = work.tile([128, 128], BDT, tag="sig_r")
                    rh = work.tile([128, 128], BDT, tag="rh")
                    for hf, ps_half in ((0, ps_ra), (1, ps_rb)):
                        for mb2 in range(2):
                            mb = hf * 2 + mb2
                            o0 = mb2 * 32
                            wcol = H + mb * 128
                            for kx in range(2):
                                nc.tensor.matmul(
                                    ps_half[:, o0 : o0 + 32],
                                    lhsT=wx_sb[:, kx, wcol : wcol + 128],
                                    rhs=xT_sb[:, kx, t * BL : t * BL + BL],
                                    start=(kx == 0),
                                    stop=False,
                                )
                            for kb in range(4):
                                nc.tensor.matmul(
                                    ps_half[:, o0 : o0 + 32],
                                    lhsT=wh_sb[:, kb, wcol : wcol + 128],
                                    rhs=hb[:, kb * 32 : kb * 32 + 32],
                                    start=False,
                                    stop=(kb == 3),
                                )
                        sl = slice(hf * 64, hf * 64 + 64)
                        nc.vector.tensor_add(r_pre[:, sl], ps_half[:], bias_r[:, sl])
                        nc.scalar.activation(sig_r[:, sl], r_pre[:, sl], ACT.Sigmoid)
                        nc.vector.tensor_mul(rh[:, sl], sig_r[:, sl], hb[:, sl])

                    gate_mms(ps_z, 0, hb, t)

                    gate_mms(ps_h, 2, rh, t)

                    z_pre = work.tile([128, 128], FDT, tag="z_pre")
                    sig_z = work.tile([128, 128], BDT, tag="sig_z")
                    nc.vector.tensor_add(z_pre[:], ps_z[:], bias_z[:])
                    nc.scalar.activation(sig_z[:], z_pre[:], ACT.Sigmoid)

                    # h~ + blend in two 64-col halves: tanh of half 1
                    # overlaps blend of half 0 (ACT and DVE in parallel)
                    h_pre = work.tile([128, 128], FDT, tag="h_pre")
                    th = work.tile([128, 128], BDT, tag="th")
                    dd = work.tile([128, 128], BDT, tag="dd")
                    ee = work.tile([128, 128], BDT, tag="ee")
                    hb_new = stateb.tile([128, 128], BDT, tag="hb")
                    for hf in range(2):
                        sl = slice(hf * 64, hf * 64 + 64)
                        nc.vector.tensor_add(h_pre[:, sl], ps_h[:, sl], bias_h[:, sl])
                        nc.scalar.activation(th[:, sl], h_pre[:, sl], ACT.Tanh)
                        nc.vector.tensor_sub(dd[:, sl], th[:, sl], hb[:, sl])
                        nc.vector.tensor_mul(ee[:, sl], dd[:, sl], sig_z[:, sl])
                        nc.vector.tensor_add(hb_new[:, sl], hb[:, sl], ee[:, sl])

                    nc.sync.dma_start(
                        out=states_d[t],
                        in_=hb_new.rearrange("p (kb b) -> p kb b", kb=4),
                    )
                    hb = hb_new

    _raw_to_json = nc.to_json_bytes
    nc.to_json_bytes = lambda: _split_multiwait_json(_raw_to_json())
    return nc


def _get_program():
    if "nc" not in _prog_cache:
        _prog_cache["nc"] = _build_program()
    return _prog_cache["nc"]


def _constrain_np(Wm):
    # column-norm constraint per (H,H) gate block, matching reference
    out = np.empty_like(Wm)
    for g in range(3):
        Wg = Wm[:, g * H : (g + 1) * H]
        norm = np.linalg.norm(Wg, axis=0, keepdims=True)
        desired = np.minimum(norm, MAX_NORM)
        out[:, g * H : (g + 1) * H] = Wg * (desired / (1e-7 + norm))
    return out


def kernel(x, Wx_f, Wh_f, b_f, Wx_b, Wh_b, b_b):
    global last_exec_time_ns
    from concourse.bass_utils import run_bass_kernel_spmd

    x = np.asarray(x, dtype=np.float32)
    Whf_c = _constrain_np(np.asarray(Wh_f, dtype=np.float32))
    Whb_c = _constrain_np(np.asarray(Wh_b, dtype=np.float32))
    wx_f = np.asarray(Wx_f, dtype=BF16)
    wx_b = np.asarray(Wx_b, dtype=BF16)
    wh_f = Whf_c.astype(BF16)
    wh_b = Whb_c.astype(BF16)
    bf = np.asarray(b_f, dtype=np.float32)
    bb = np.asarray(b_b, dtype=np.float32)

    in_maps = []
    for c in range(N_CORES):
        d, s = divmod(c, N_BSHARD)
        xs = x[s * BL : (s + 1) * BL]
        if d == 1:
            xs = xs[:, ::-1]
        xTc = np.ascontiguousarray(xs.transpose(2, 1, 0)).reshape(I, T * BL)
        in_maps.append(
            {
                "xT": xTc.astype(BF16),
                "wx": wx_f if d == 0 else wx_b,
                "wh": wh_f if d == 0 else wh_b,
                "bias": bf if d == 0 else bb,
            }
        )

    nc = _get_program()
    trace = os.environ.get("BASS_TRACE") == "1"
    res = run_bass_kernel_spmd(nc, in_maps, list(range(N_CORES)), trace=trace)
    last_exec_time_ns = res.exec_time_ns

    def to_bth(st):
        # (T, 128, 4, BL) -> (BL, T, H) with H = kb*128 + p
        return np.ascontiguousarray(
            st.astype(np.float32).transpose(3, 0, 2, 1).reshape(BL, T, H)
        )

    fwd = np.concatenate([to_bth(res.results[c]["states"]) for c in range(4)], axis=0)
    bwd = np.concatenate(
        [to_bth(res.results[c + 4]["states"]) for c in range(4)], axis=0
    )[:, ::-1]
    combined = np.concatenate([fwd[:, -1], bwd[:, 0]], axis=-1)
    return (
        np.ascontiguousarray(fwd),
        np.ascontiguousarray(bwd),
        np.ascontiguousarray(combined),
    )


# revision 32
# speedup vs baseline: 1.1171x; 1.0889x over previous
"""Bidirectional GRU on 8 Trainium2 NeuronCores.

Problem: x (128, 512, 256), GRU hidden 512, two directions, column-norm
constrained Wh gate blocks. Sharding: 2 directions x 4 batch slices of 32.
All 8 cores run an identical SPMD program (forward GRU); the backward
direction is realized by time-flipping x on the host for cores 4-7 and
un-flipping their output states.

Kernel layout (per core): state is kept transposed (hT: H on partitions,
batch on the free axis) so the recurrent matmuls use the resident Wh blocks
as stationary operands and no per-step transposes are needed.
Phase 1 precomputes xpT = Wx^T @ xT + b for all steps (bf16, PE) into DRAM;
phase 2 runs the 512-step recurrence with fp32 state and bf16 matmul
operands.
"""

import os

import numpy as np
import ml_dtypes

B, T, I, H = 128, int(os.environ.get("GRU_T", "512")), 256, 512
G3 = 3 * H  # 1536
N_CORES = 8
N_BSHARD = 4
BL = B // N_BSHARD  # 32 batch per core
MAX_NORM = 1.0

BF16 = ml_dtypes.bfloat16

_prog_cache = {}
last_exec_time_ns = None


# ---------------------------------------------------------------------------
# Tile drain patch: this container's walrus build rejects instructions that
# carry too many sem waits ("Too many sync wait commands" in CoreV3 codegen).
# Spread the final Tile drain's waits across preceding sync-engine nops.
# ---------------------------------------------------------------------------
def _install_tile_drain_patch():
    import concourse.mybir as mybir
    import concourse.tile as tile
    from concourse.vector_clock import ScopedClock

    if getattr(tile.TileContext, "_drain_patch_installed", False):
        return

    max_waits = 1

    def _patched_drain_and_barrier(self, tick_clock, wait_clock):
        nc = self.nc
        lead = nc.sync.nop(nofuse=True)
        wait_clock.add_sem_waits(
            lead.ins, ScopedClock({None: tick_clock.global_clock})
        )
        si = lead.ins.sync_info
        waits = list(si.on_wait or []) if si is not None else []
        if len(waits) > max_waits:
            si.on_wait = waits[:max_waits]
            for i in range(max_waits, len(waits), max_waits):
                nop = nc.sync.nop(nofuse=True)
                nop.ins.sync_info = mybir.SyncInfo(
                    on_wait=waits[i : i + max_waits], on_update=[]
                )
        nc.sync.drain()

        nc.all_engine_barrier()
        assert self.sems is not None
        popped = nc._tile_sem_poison_stack.pop()
        assert popped is self._sem_poison
        nc.clear_and_free_semaphores(list(self.sems.allocated().values()))
        nc.all_engine_barrier()

    tile.TileContext._drain_and_barrier = _patched_drain_and_barrier
    tile.TileContext._drain_patch_installed = True


def _split_multiwait_json(bir_bytes, max_waits=1):
    """This walrus build rejects instructions carrying more than one sem wait.
    Hoist extra waits onto same-engine NoOps inserted just before the
    instruction (engines execute their queue in order, so semantics hold)."""
    import json

    bir = json.loads(bir_bytes)
    ctr = 0
    for f in bir["functions"]:
        for blk in f["blocks"]:
            out = []
            for inst in blk["instructions"]:
                si = inst.get("sync_info")
                ow = (si or {}).get("on_wait") or []
                if len(ow) > max_waits:
                    extra = ow[: len(ow) - max_waits]
                    for i in range(0, len(extra), max_waits):
                        ctr += 1
                        out.append(
                            {
                                "debug": 0,
                                "engine": inst["engine"],
                                "ins": [],
                                "outs": [],
                                "name": f"I-wn{ctr}",
                                "opcode": "NoOp",
                                "sync_info": {
                                    "on_update": [],
                                    "on_wait": extra[i : i + max_waits],
                                },
                            }
                        )
                    si["on_wait"] = ow[len(ow) - max_waits :]
                out.append(inst)
            blk["instructions"] = out
    return json.dumps(bir).encode()


def _build_program():
    import concourse.bass as bass
    import concourse.mybir as mybir
    import concourse.tile as tile

    _install_tile_drain_patch()

    FDT = mybir.dt.float32
    BDT = mybir.dt.bfloat16
    ACT = mybir.ActivationFunctionType
    NTOK = T * BL  # 16384

    nc = bass.Bass()
    xT_d = nc.dram_tensor("xT", [I, NTOK], BDT, kind="ExternalInput")
    wx_d = nc.dram_tensor("wx", [I, G3], BDT, kind="ExternalInput")
    wh_d = nc.dram_tensor("wh", [H, G3], BDT, kind="ExternalInput")
    bias_d = nc.dram_tensor("bias", [G3], FDT, kind="ExternalInput")
    states_d = nc.dram_tensor("states", [T, 128, 4, BL], BDT, kind="ExternalOutput")

    with tile.TileContext(nc) as tc:
        with (
            tc.tile_pool(name="const", bufs=1) as const,
            tc.tile_pool(name="xtp", bufs=1) as xtp,
        ):
            wx_sb = const.tile([128, 2, G3], BDT)
            nc.sync.dma_start(
                out=wx_sb[:], in_=wx_d.rearrange("(kb p) m -> p kb m", p=128)
            )
            wh_sb = const.tile([128, 4, G3], BDT)
            nc.sync.dma_start(
                out=wh_sb[:], in_=wh_d.rearrange("(kb p) m -> p kb m", p=128)
            )
            xT_sb = xtp.tile([128, 2, NTOK], BDT)
            nc.sync.dma_start(
                out=xT_sb[:], in_=xT_d.rearrange("(kb p) n -> p kb n", p=128)
            )

            bias_sb = const.tile([128, 12], FDT)
            nc.sync.dma_start(
                out=bias_sb[:], in_=bias_d.rearrange("(m p) -> p m", p=128)
            )
            # broadcast bias tiles: bias_g[p, mb*32+b] = b[g*512+mb*128+p]
            bias_r = const.tile([128, 128], FDT, tag="bias_r")
            bias_z = const.tile([128, 128], FDT, tag="bias_z")
            bias_h = const.tile([128, 128], FDT, tag="bias_h")
            for bt, g0 in ((bias_z, 0), (bias_r, 4), (bias_h, 8)):
                nc.vector.memset(bt[:], 0.0)
                for mb in range(4):
                    sl = slice(mb * 32, mb * 32 + 32)
                    nc.vector.tensor_scalar_add(
                        bt[:, sl], bt[:, sl], bias_sb[:, g0 + mb : g0 + mb + 1]
                    )

            # ---------------- recurrence ------------------------------------
            # Per step, per gate slice mb: PSUM group = [x-proj (2 matmuls,
            # no state dep) then 4 recurrent matmuls]. r gate first so its
            # sigmoid/mul overlap the z matmuls; h~ after rh is ready.
            with (
                tc.tile_pool(name="state", bufs=3) as state,
                tc.tile_pool(name="stateb", bufs=3) as stateb,
                tc.tile_pool(name="work", bufs=3) as work,
                tc.tile_pool(name="p_r", bufs=2, space="PSUM") as p_r,
                tc.tile_pool(name="p_z", bufs=1, space="PSUM") as p_z,
                tc.tile_pool(name="p_h", bufs=1, space="PSUM") as p_h,
            ):
                hb = stateb.tile([128, 128], BDT, tag="hb")
                nc.vector.memset(hb[:], 0.0)

                def gate_mms(ps, g, rhs_state, t):
                    for mb in range(4):
                        o0 = mb * 32
                        wcol = g * H + mb * 128
                        for kx in range(2):
                            nc.tensor.matmul(
                                ps[:, o0 : o0 + 32],
                                lhsT=wx_sb[:, kx, wcol : wcol + 128],
                                rhs=xT_sb[:, kx, t * BL : t * BL + BL],
                                start=(kx == 0),
                                stop=False,
                            )
                        for kb in range(4):
                            nc.tensor.matmul(
                                ps[:, o0 : o0 + 32],
                                lhsT=wh_sb[:, kb, wcol : wcol + 128],
                                rhs=rhs_state[:, kb * 32 : kb * 32 + 32],
                                start=False,
                                stop=(kb == 3),
                            )

                for t in range(T):
                    ps_ra = p_r.tile([128, 64], FDT, tag="ps_ra")
                    ps_rb = p_r.tile([128, 64], FDT, tag="ps_rb")
                    ps_z = p_z.tile([128, 128], FDT, tag="ps_z")
                    ps_ha = p_h.tile([128, 64], FDT, tag="ps_ha")
                    ps_hb = p_h.tile([128, 64], FDT, tag="ps_hb")

                    # r gate in two PSUM tiles: the add/sigmoid/mul chain of
                    # half A pipelines with half B's matmuls (per-tile deps)
                    r_pre = work.tile([128, 128], FDT, tag="r_pre")
                    sig_r = work.tile([128, 128], BDT, tag="sig_r")
                    rh = work.tile([128, 128], BDT, tag="rh")
                    for hf, ps_half in ((0, ps_ra), (1, ps_rb)):
                        for mb2 in range(2):
                            mb = hf * 2 + mb2
                            o0 = mb2 * 32
                            wcol = H + mb * 128
                            for kx in range(2):
                                nc.tensor.matmul(
                                    ps_half[:, o0 : o0 + 32],
                                    lhsT=wx_sb[:, kx, wcol : wcol + 128],
                                    rhs=xT_sb[:, kx, t * BL : t * BL + BL],
                                    start=(kx == 0),
                                    stop=False,
                                )
                            for kb in range(4):
                                nc.tensor.matmul(
                                    ps_half[:, o0 : o0 + 32],
                                    lhsT=wh_sb[:, kb, wcol : wcol + 128],
                                    rhs=hb[:, kb * 32 : kb * 32 + 32],
                                    start=False,
                                    stop=(kb == 3),
                                )
                        sl = slice(hf * 64, hf * 64 + 64)
                        nc.vector.tensor_add(r_pre[:, sl], ps_half[:], bias_r[:, sl])
                        nc.scalar.activation(sig_r[:, sl], r_pre[:, sl], ACT.Sigmoid)
                        nc.vector.tensor_mul(rh[:, sl], sig_r[:, sl], hb[:, sl])

                    gate_mms(ps_z, 0, hb, t)

                    for hf, ps_half in ((0, ps_ha), (1, ps_hb)):
                        for mb2 in range(2):
                            mb = hf * 2 + mb2
                            o0 = mb2 * 32
                            wcol = 2 * H + mb * 128
                            for kx in range(2):
                                nc.tensor.matmul(
                                    ps_half[:, o0 : o0 + 32],
                                    lhsT=wx_sb[:, kx, wcol : wcol + 128],
                                    rhs=xT_sb[:, kx, t * BL : t * BL + BL],
                                    start=(kx == 0),
                                    stop=False,
                                )
                            for kb in range(4):
                                nc.tensor.matmul(
                                    ps_half[:, o0 : o0 + 32],
                                    lhsT=wh_sb[:, kb, wcol : wcol + 128],
                                    rhs=rh[:, kb * 32 : kb * 32 + 32],
                                    start=False,
                                    stop=(kb == 3),
                                )

                    z_pre = work.tile([128, 128], FDT, tag="z_pre")
                    sig_z = work.tile([128, 128], BDT, tag="sig_z")
                    nc.vector.tensor_add(z_pre[:], ps_z[:], bias_z[:])
                    nc.scalar.activation(sig_z[:], z_pre[:], ACT.Sigmoid)

                    # h~ + blend in two 64-col halves: tanh of half 1
                    # overlaps blend of half 0 (ACT and DVE in parallel)
                    h_pre = work.tile([128, 128], FDT, tag="h_pre")
                    th = work.tile([128, 128], BDT, tag="th")
                    dd = work.tile([128, 128], BDT, tag="dd")
                    ee = work.tile([128, 128], BDT, tag="ee")
                    hb_new = stateb.tile([128, 128], BDT, tag="hb")
                    for hf, ps_half in ((0, ps_ha), (1, ps_hb)):
                        sl = slice(hf * 64, hf * 64 + 64)
                        nc.vector.tensor_add(h_pre[:, sl], ps_half[:], bias_h[:, sl])
                        nc.scalar.activation(th[:, sl], h_pre[:, sl], ACT.Tanh)
                        nc.vector.tensor_sub(dd[:, sl], th[:, sl], hb[:, sl])
                        nc.vector.tensor_mul(ee[:, sl], dd[:, sl], sig_z[:, sl])
                        nc.vector.tensor_add(hb_new[:, sl], hb[:, sl], ee[:, sl])

                    nc.sync.dma_start(
                        out=states_d[t],
                        in_=hb_new.rearrange("p (kb b) -> p kb b", kb=4),
                    )
                    hb = hb_new

    _raw_to_json = nc.to_json_bytes
    nc.to_json_bytes = lambda: _split_multiwait_json(_raw_to_json())
    return nc


def _get_program():
    if "nc" not in _prog_cache:
        _prog_cache["nc"] = _build_program()
    return _prog_cache["nc"]


def _constrain_np(Wm):
    # column-norm constraint per (H,H) gate block, matching reference
    out = np.empty_like(Wm)
    for g in range(3):
        Wg = Wm[:, g * H : (g + 1) * H]
        norm = np.linalg.norm(Wg, axis=0, keepdims=True)
        desired = np.minimum(norm, MAX_NORM)
        out[:, g * H : (g + 1) * H] = Wg * (desired / (1e-7 + norm))
    return out


def kernel(x, Wx_f, Wh_f, b_f, Wx_b, Wh_b, b_b):
    global last_exec_time_ns
    from concourse.bass_utils import run_bass_kernel_spmd

    x = np.asarray(x, dtype=np.float32)
    Whf_c = _constrain_np(np.asarray(Wh_f, dtype=np.float32))
    Whb_c = _constrain_np(np.asarray(Wh_b, dtype=np.float32))
    wx_f = np.asarray(Wx_f, dtype=BF16)
    wx_b = np.asarray(Wx_b, dtype=BF16)
    wh_f = Whf_c.astype(BF16)
    wh_b = Whb_c.astype(BF16)
    bf = np.asarray(b_f, dtype=np.float32)
    bb = np.asarray(b_b, dtype=np.float32)

    in_maps = []
    for c in range(N_CORES):
        d, s = divmod(c, N_BSHARD)
        xs = x[s * BL : (s + 1) * BL]
        if d == 1:
            xs = xs[:, ::-1]
        xTc = np.ascontiguousarray(xs.transpose(2, 1, 0)).reshape(I, T * BL)
        in_maps.append(
            {
                "xT": xTc.astype(BF16),
                "wx": wx_f if d == 0 else wx_b,
                "wh": wh_f if d == 0 else wh_b,
                "bias": bf if d == 0 else bb,
            }
        )

    nc = _get_program()
    trace = os.environ.get("BASS_TRACE") == "1"
    res = run_bass_kernel_spmd(nc, in_maps, list(range(N_CORES)), trace=trace)
    last_exec_time_ns = res.exec_time_ns

    def to_bth(st):
        # (T, 128, 4, BL) -> (BL, T, H) with H = kb*128 + p
        return np.ascontiguousarray(
            st.astype(np.float32).transpose(3, 0, 2, 1).reshape(BL, T, H)
        )

    fwd = np.concatenate([to_bth(res.results[c]["states"]) for c in range(4)], axis=0)
    bwd = np.concatenate(
        [to_bth(res.results[c + 4]["states"]) for c in range(4)], axis=0
    )[:, ::-1]
    combined = np.concatenate([fwd[:, -1], bwd[:, 0]], axis=-1)
    return (
        np.ascontiguousarray(fwd),
        np.ascontiguousarray(bwd),
        np.ascontiguousarray(combined),
    )


# revision 34
# speedup vs baseline: 1.1197x; 1.0024x over previous
"""Bidirectional GRU on 8 Trainium2 NeuronCores.

Problem: x (128, 512, 256), GRU hidden 512, two directions, column-norm
constrained Wh gate blocks. Sharding: 2 directions x 4 batch slices of 32.
All 8 cores run an identical SPMD program (forward GRU); the backward
direction is realized by time-flipping x on the host for cores 4-7 and
un-flipping their output states.

Kernel layout (per core): state is kept transposed (hT: H on partitions,
batch on the free axis) so the recurrent matmuls use the resident Wh blocks
as stationary operands and no per-step transposes are needed.
Phase 1 precomputes xpT = Wx^T @ xT + b for all steps (bf16, PE) into DRAM;
phase 2 runs the 512-step recurrence with fp32 state and bf16 matmul
operands.
"""

import os

import numpy as np
import ml_dtypes

B, T, I, H = 128, int(os.environ.get("GRU_T", "512")), 256, 512
G3 = 3 * H  # 1536
N_CORES = 8
N_BSHARD = 4
BL = B // N_BSHARD  # 32 batch per core
MAX_NORM = 1.0

BF16 = ml_dtypes.bfloat16

_prog_cache = {}
last_exec_time_ns = None


# ---------------------------------------------------------------------------
# Tile drain patch: this container's walrus build rejects instructions that
# carry too many sem waits ("Too many sync wait commands" in CoreV3 codegen).
# Spread the final Tile drain's waits across preceding sync-engine nops.
# ---------------------------------------------------------------------------
def _install_tile_drain_patch():
    import concourse.mybir as mybir
    import concourse.tile as tile
    from concourse.vector_clock import ScopedClock

    if getattr(tile.TileContext, "_drain_patch_installed", False):
        return

    max_waits = 1

    def _patched_drain_and_barrier(self, tick_clock, wait_clock):
        nc = self.nc
        lead = nc.sync.nop(nofuse=True)
        wait_clock.add_sem_waits(
            lead.ins, ScopedClock({None: tick_clock.global_clock})
        )
        si = lead.ins.sync_info
        waits = list(si.on_wait or []) if si is not None else []
        if len(waits) > max_waits:
            si.on_wait = waits[:max_waits]
            for i in range(max_waits, len(waits), max_waits):
                nop = nc.sync.nop(nofuse=True)
                nop.ins.sync_info = mybir.SyncInfo(
                    on_wait=waits[i : i + max_waits], on_update=[]
                )
        nc.sync.drain()

        nc.all_engine_barrier()
        assert self.sems is not None
        popped = nc._tile_sem_poison_stack.pop()
        assert popped is self._sem_poison
        nc.clear_and_free_semaphores(list(self.sems.allocated().values()))
        nc.all_engine_barrier()

    tile.TileContext._drain_and_barrier = _patched_drain_and_barrier
    tile.TileContext._drain_patch_installed = True


def _split_multiwait_json(bir_bytes, max_waits=1):
    """This walrus build rejects instructions carrying more than one sem wait.
    Hoist extra waits onto same-engine NoOps inserted just before the
    instruction (engines execute their queue in order, so semantics hold)."""
    import json

    bir = json.loads(bir_bytes)
    ctr = 0
    for f in bir["functions"]:
        for blk in f["blocks"]:
            out = []
            for inst in blk["instructions"]:
                si = inst.get("sync_info")
                ow = (si or {}).get("on_wait") or []
                if len(ow) > max_waits:
                    extra = ow[: len(ow) - max_waits]
                    for i in range(0, len(extra), max_waits):
                        ctr += 1
                        out.append(
                            {
                                "debug": 0,
                                "engine": inst["engine"],
                                "ins": [],
                                "outs": [],
                                "name": f"I-wn{ctr}",
                                "opcode": "NoOp",
                                "sync_info": {
                                    "on_update": [],
                                    "on_wait": extra[i : i + max_waits],
                                },
                            }
                        )
                    si["on_wait"] = ow[len(ow) - max_waits :]
                out.append(inst)
            blk["instructions"] = out
    return json.dumps(bir).encode()


def _build_program():
    import concourse.bass as bass
    import concourse.mybir as mybir
    import concourse.tile as tile

    _install_tile_drain_patch()

    FDT = mybir.dt.float32
    BDT = mybir.dt.bfloat16
    ACT = mybir.ActivationFunctionType
    NTOK = T * BL  # 16384

    nc = bass.Bass()
    xT_d = nc.dram_tensor("xT", [I, NTOK], BDT, kind="ExternalInput")
    wx_d = nc.dram_tensor("wx", [I, G3], BDT, kind="ExternalInput")
    wh_d = nc.dram_tensor("wh", [H, G3], BDT, kind="ExternalInput")
    bias_d = nc.dram_tensor("bias", [G3], FDT, kind="ExternalInput")
    states_d = nc.dram_tensor("states", [T, 128, 4, BL], BDT, kind="ExternalOutput")

    with tile.TileContext(nc) as tc:
        with (
            tc.tile_pool(name="const", bufs=1) as const,
            tc.tile_pool(name="xtp", bufs=1) as xtp,
        ):
            wx_sb = const.tile([128, 2, G3], BDT)
            nc.sync.dma_start(
                out=wx_sb[:], in_=wx_d.rearrange("(kb p) m -> p kb m", p=128)
            )
            wh_sb = const.tile([128, 4, G3], BDT)
            nc.sync.dma_start(
                out=wh_sb[:], in_=wh_d.rearrange("(kb p) m -> p kb m", p=128)
            )
            xT_sb = xtp.tile([128, 2, NTOK], BDT)
            nc.sync.dma_start(
                out=xT_sb[:], in_=xT_d.rearrange("(kb p) n -> p kb n", p=128)
            )

            bias_sb = const.tile([128, 12], FDT)
            nc.sync.dma_start(
                out=bias_sb[:], in_=bias_d.rearrange("(m p) -> p m", p=128)
            )
            # broadcast bias tiles: bias_g[p, mb*32+b] = b[g*512+mb*128+p]
            bias_r = const.tile([128, 128], FDT, tag="bias_r")
            bias_z = const.tile([128, 128], FDT, tag="bias_z")
            bias_h = const.tile([128, 128], FDT, tag="bias_h")
            for bt, g0 in ((bias_z, 0), (bias_r, 4), (bias_h, 8)):
                nc.vector.memset(bt[:], 0.0)
                for mb in range(4):
                    sl = slice(mb * 32, mb * 32 + 32)
                    nc.vector.tensor_scalar_add(
                        bt[:, sl], bt[:, sl], bias_sb[:, g0 + mb : g0 + mb + 1]
                    )

            # ---------------- recurrence ------------------------------------
            # Per step, per gate slice mb: PSUM group = [x-proj (2 matmuls,
            # no state dep) then 4 recurrent matmuls]. r gate first so its
            # sigmoid/mul overlap the z matmuls; h~ after rh is ready.
            with (
                tc.tile_pool(name="state", bufs=3) as state,
                tc.tile_pool(name="stateb", bufs=3) as stateb,
                tc.tile_pool(name="work", bufs=3) as work,
                tc.tile_pool(name="p_r", bufs=2, space="PSUM") as p_r,
                tc.tile_pool(name="p_z", bufs=1, space="PSUM") as p_z,
                tc.tile_pool(name="p_h", bufs=1, space="PSUM") as p_h,
            ):
                hb = stateb.tile([128, 128], BDT, tag="hb")
                nc.vector.memset(hb[:], 0.0)

                def gate_mms(ps, g, rhs_state, t):
                    for mb in range(4):
                        o0 = mb * 32
                        wcol = g * H + mb * 128
                        for kx in range(2):
                            nc.tensor.matmul(
                                ps[:, o0 : o0 + 32],
                                lhsT=wx_sb[:, kx, wcol : wcol + 128],
                                rhs=xT_sb[:, kx, t * BL : t * BL + BL],
                                start=(kx == 0),
                                stop=False,
                            )
                        for kb in range(4):
                            nc.tensor.matmul(
                                ps[:, o0 : o0 + 32],
                                lhsT=wh_sb[:, kb, wcol : wcol + 128],
                                rhs=rhs_state[:, kb * 32 : kb * 32 + 32],
                                start=False,
                                stop=(kb == 3),
                            )

                for t in range(T):
                    ps_ra = p_r.tile([128, 64], FDT, tag="ps_ra")
                    ps_rb = p_r.tile([128, 64], FDT, tag="ps_rb")
                    ps_z = p_z.tile([128, 128], FDT, tag="ps_z")
                    ps_ha = p_h.tile([128, 64], FDT, tag="ps_ha")
                    ps_hb = p_h.tile([128, 64], FDT, tag="ps_hb")

                    # r gate in two PSUM tiles: the add/sigmoid/mul chain of
                    # half A pipelines with half B's matmuls (per-tile deps)
                    r_pre = work.tile([128, 128], FDT, tag="r_pre")
                    sig_r = work.tile([128, 128], BDT, tag="sig_r")
                    rh = work.tile([128, 128], BDT, tag="rh")
                    for hf, ps_half in ((0, ps_ra), (1, ps_rb)):
                        for mb2 in range(2):
                            mb = hf * 2 + mb2
                            o0 = mb2 * 32
                            wcol = H + mb * 128
                            for kx in range(2):
                                nc.tensor.matmul(
                                    ps_half[:, o0 : o0 + 32],
                                    lhsT=wx_sb[:, kx, wcol : wcol + 128],
                                    rhs=xT_sb[:, kx, t * BL : t * BL + BL],
                                    start=(kx == 0),
                                    stop=False,
                                )
                            for kb in range(4):
                                nc.tensor.matmul(
                                    ps_half[:, o0 : o0 + 32],
                                    lhsT=wh_sb[:, kb, wcol : wcol + 128],
                                    rhs=hb[:, kb * 32 : kb * 32 + 32],
                                    start=False,
                                    stop=(kb == 3),
                                )
                        sl = slice(hf * 64, hf * 64 + 64)
                        nc.vector.tensor_add(r_pre[:, sl], ps_half[:], bias_r[:, sl])
                        nc.scalar.activation(sig_r[:, sl], r_pre[:, sl], ACT.Sigmoid)
                        nc.vector.tensor_mul(rh[:, sl], sig_r[:, sl], hb[:, sl])

                    gate_mms(ps_z, 0, hb, t)

                    for hf, ps_half in ((0, ps_ha), (1, ps_hb)):
                        for mb2 in range(2):
                            mb = hf * 2 + mb2
                            o0 = mb2 * 32
                            wcol = 2 * H + mb * 128
                            for kx in range(2):
                                nc.tensor.matmul(
                                    ps_half[:, o0 : o0 + 32],
                                    lhsT=wx_sb[:, kx, wcol : wcol + 128],
                                    rhs=xT_sb[:, kx, t * BL : t * BL + BL],
                                    start=(kx == 0),
                                    stop=False,
                                )
                            for kb in range(4):
                                nc.tensor.matmul(
                                    ps_half[:, o0 : o0 + 32],
                                    lhsT=wh_sb[:, kb, wcol : wcol + 128],
                                    rhs=rh[:, kb * 32 : kb * 32 + 32],
                                    start=False,
                                    stop=(kb == 3),
                                )

                    z_pre = work.tile([128, 128], FDT, tag="z_pre")
                    sig_z = work.tile([128, 128], BDT, tag="sig_z")
                    nc.vector.tensor_add(z_pre[:], ps_z[:], bias_z[:])
                    nc.scalar.activation(sig_z[:], z_pre[:], ACT.Sigmoid)

                    # h~ + blend in two 64-col halves: tanh of half 1
                    # overlaps blend of half 0 (ACT and DVE in parallel)
                    h_pre = work.tile([128, 128], FDT, tag="h_pre")
                    th = work.tile([128, 128], BDT, tag="th")
                    dd = work.tile([128, 128], BDT, tag="dd")
                    ee = work.tile([128, 128], BDT, tag="ee")
                    hb_new = stateb.tile([128, 128], BDT, tag="hb")
                    for hf, ps_half in ((0, ps_ha), (1, ps_hb)):
                        sl = slice(hf * 64, hf * 64 + 64)
                        nc.vector.tensor_add(h_pre[:, sl], ps_half[:], bias_h[:, sl])
                        nc.scalar.activation(th[:, sl], h_pre[:, sl], ACT.Tanh)
                        nc.vector.tensor_sub(dd[:, sl], th[:, sl], hb[:, sl])
                        nc.vector.tensor_mul(ee[:, sl], dd[:, sl], sig_z[:, sl])
                        nc.vector.tensor_add(hb_new[:, sl], hb[:, sl], ee[:, sl])

                    nc.sync.dma_start(
                        out=states_d[t],
                        in_=hb_new.rearrange("p (kb b) -> p kb b", kb=4),
                    )
                    hb = hb_new

    _raw_to_json = nc.to_json_bytes
    nc.to_json_bytes = lambda: _split_multiwait_json(_raw_to_json())
    return nc


def _get_program():
    if "nc" not in _prog_cache:
        _prog_cache["nc"] = _build_program()
    return _prog_cache["nc"]


def _constrain_np(Wm):
    # column-norm constraint per (H,H) gate block, matching reference
    out = np.empty_like(Wm)
    for g in range(3):
        Wg = Wm[:, g * H : (g + 1) * H]
        norm = np.linalg.norm(Wg, axis=0, keepdims=True)
        desired = np.minimum(norm, MAX_NORM)
        out[:, g * H : (g + 1) * H] = Wg * (desired / (1e-7 + norm))
    return out


def kernel(x, Wx_f, Wh_f, b_f, Wx_b, Wh_b, b_b):
    global last_exec_time_ns
    from concourse.bass_utils import run_bass_kernel_spmd

    x = np.asarray(x, dtype=np.float32)
    Whf_c = _constrain_np(np.asarray(Wh_f, dtype=np.float32))
    Whb_c = _constrain_np(np.asarray(Wh_b, dtype=np.float32))
    wx_f = np.asarray(Wx_f, dtype=BF16)
    wx_b = np.asarray(Wx_b, dtype=BF16)
    wh_f = Whf_c.astype(BF16)
    wh_b = Whb_c.astype(BF16)
    bf = np.asarray(b_f, dtype=np.float32)
    bb = np.asarray(b_b, dtype=np.float32)

    in_maps = []
    for c in range(N_CORES):
        d, s = divmod(c, N_BSHARD)
        xs = x[s * BL : (s + 1) * BL]
        if d == 1:
            xs = xs[:, ::-1]
        xTc = np.ascontiguousarray(xs.transpose(2, 1, 0)).reshape(I, T * BL)
        in_maps.append(
            {
                "xT": xTc.astype(BF16),
                "wx": wx_f if d == 0 else wx_b,
                "wh": wh_f if d == 0 else wh_b,
                "bias": bf if d == 0 else bb,
            }
        )

    nc = _get_program()
    trace = os.environ.get("BASS_TRACE") == "1"
    res = run_bass_kernel_spmd(nc, in_maps, list(range(N_CORES)), trace=trace)
    last_exec_time_ns = res.exec_time_ns

    def to_bth(st):
        # (T, 128, 4, BL) -> (BL, T, H) with H = kb*128 + p
        return np.ascontiguousarray(
            st.astype(np.float32).transpose(3, 0, 2, 1).reshape(BL, T, H)
        )

    fwd = np.concatenate([to_bth(res.results[c]["states"]) for c in range(4)], axis=0)
    bwd = np.concatenate(
        [to_bth(res.results[c + 4]["states"]) for c in range(4)], axis=0
    )[:, ::-1]
    combined = np.concatenate([fwd[:, -1], bwd[:, 0]], axis=-1)
    return (
        np.ascontiguousarray(fwd),
        np.ascontiguousarray(bwd),
        np.ascontiguousarray(combined),
    )
